# revision 1
# baseline (speedup 1.0000x reference)
"""Trainium2 Bass kernel for nn_EncoderLayer (multiplicative-attention encoder layer).

Sharding: 8 cores; core c handles batch b=c//2, head-group hg=c%2 (4 of 8 heads).
The reference's head-major reshape bug maps head h exactly to output rows
[256h, 256h+256), so each core owns 1024 complete output rows -> no collectives.

Device dataflow (all activations transposed [feature, seq] in SBUF):
  Q^T = (Wm@Wq) @ src^T + (Wm bq + bm)    (Wm folded on host)
  K^T/V from host-compacted unmasked keys (src_mask gather, zero-pad to 128)
  energy^T[k,q] tiles -> ACT exp(e + maskbias[k]) -> attn^T bf16
  x'^T[65,q] += V'[k,65]^T @ attn^T (col 64 of V' is ones -> softmax denominator)
  xn^T = x'^T * recip(den) (recip broadcast via K=1 ones-matmul on PE)
  Wo via strided rhs views of xn (the scrambled reshape is a free AP), +src residual
  LN1 (stats via ones-matmuls; rsqrt = exp(-0.5 ln(v+eps))) -> FFN -> LN2 -> out^T
Host reassembles out[b, 1024*hg : 1024*(hg+1), :] = out_t.T per core.
"""

import math

import numpy as np
import ml_dtypes

import concourse.bass as bass
import concourse.tile as tile
import concourse.bacc as bacc
from concourse import mybir
from concourse import bass_utils
from concourse import hw_specs as _hw_specs

_real_gat = _hw_specs.get_activation_tables


def _gat_pinned(arch):
    tabs = _real_gat(arch)
    return {name: (fns if name == "natural_log_exp_and_others" else set())
            for name, fns in tabs.items()}


bacc.get_activation_tables = _gat_pinned

B, S, HID, H, PF, D = 4, 2048, 512, 8, 2048, 64
N_CORES = 8
HPC = H // 2          # heads per core (4)
R = HPC * 256         # output rows per core (1024)
F32 = mybir.dt.float32
BF16 = mybir.dt.bfloat16
AF = mybir.ActivationFunctionType
OP = mybir.AluOpType
NEG_BIG = -87.0       # exp(-87) ~ 1e-38 ~ 0 in fp32, still inside ACT exp table domain
LN_EPS = 1e-5

_built_cache = {}
last_results = None   # BassKernelResults of the most recent run (for test harness)
run_kwargs = {}       # extra kwargs for run_bass_kernel_spmd (e.g. trace=True)


def _bcast_ap(ap_1d, parts):
    """[N] DRAM AP -> [parts, N] partition-broadcast AP (stride-0 partition dim)."""
    return bass.AP(tensor=ap_1d.tensor, offset=ap_1d.offset,
                   ap=[[0, parts], *ap_1d.ap])


def _bcast_row(ap_2d, parts):
    """[1, N] AP -> [parts, N] partition-broadcast AP."""
    return bass.AP(tensor=ap_2d.tensor, offset=ap_2d.offset,
                   ap=[[0, parts], ap_2d.ap[-1]])


def build_bass(sup):
    """Build the per-core Bass module. sup = padded unmasked key count (mult of 128)."""
    KT = sup // 128
    nc = bacc.Bacc("TRN2", target_bir_lowering=False, debug=False,
                   num_devices=N_CORES)

    def inp(name, shape, dt=F32):
        return nc.dram_tensor(name, shape, dt, kind="ExternalInput").ap()

    src_bf_d = inp("src_bf", [HID, S], BF16)     # src[b].T bf16
    src_res_d = inp("src_res", [HID, R])         # fp32 residual slice of src[b].T
    srcu_d = inp("srcu", [HID, sup], BF16)       # gathered unmasked src[b].T, zero-pad
    wq_d = inp("wq", [HID, 2 * 128], BF16)       # (Wm@Wq_heads).T
    wk_d = inp("wk", [HID, 2 * 128], BF16)
    wv_d = inp("wv", [HID, 2 * 128], BF16)
    wo_d = inp("wo", [HID, HID], BF16)           # Wo.T
    w1_d = inp("w1", [HID, PF], BF16)            # W1.T
    w2_d = inp("w2", [PF, HID], BF16)            # W2.T
    bq_d = inp("bq", [2, 128])                   # per-partition bias packs [mt, p]
    bk_d = inp("bk", [2, 128])
    bv_d = inp("bv", [256])                      # free-dim bias (broadcast across parts)
    bo_d = inp("bo", [4, 128])
    b1_d = inp("b1", [16, 128])
    b2_d = inp("b2", [4, 128])
    g1_d = inp("g1", [4, 128])
    bt1_d = inp("bt1", [4, 128])
    g2_d = inp("g2", [4, 128])
    bt2_d = inp("bt2", [4, 128])
    mb_d = inp("mb", [KT, 128])                  # mask bias (0 real / -87 pad)
    out_d = nc.dram_tensor("out_t", [HID, R], F32, kind="ExternalOutput").ap()

    from contextlib import ExitStack
    with tile.TileContext(nc) as tc, ExitStack() as ctx:
        con = ctx.enter_context(tc.tile_pool(name="con", bufs=1))
        ppool = ctx.enter_context(tc.tile_pool(name="ps", bufs=2, space="PSUM"))
        pe_e = ctx.enter_context(tc.tile_pool(name="pe", bufs=2, space="PSUM"))
        pe_av = ctx.enter_context(tc.tile_pool(name="pav", bufs=2, space="PSUM"))
        att_pool = ctx.enter_context(tc.tile_pool(name="att", bufs=3))
        xp_pool = ctx.enter_context(tc.tile_pool(name="xp", bufs=1))
        xn_pool = ctx.enter_context(tc.tile_pool(name="xn", bufs=2))
        rep_pool = ctx.enter_context(tc.tile_pool(name="rep", bufs=2))
        h1_pool = ctx.enter_context(tc.tile_pool(name="h1", bufs=2))
        tmp_pool = ctx.enter_context(tc.tile_pool(name="tmp", bufs=2))
        z_pool = ctx.enter_context(tc.tile_pool(name="z", bufs=2))
        o_pool = ctx.enter_context(tc.tile_pool(name="o", bufs=4))

        mm = nc.tensor.matmul
        act = nc.scalar.activation
        dve = nc.vector

        def dma(out, in_):
            nc.sync.dma_start(out=out, in_=in_)

        def ctile(shape, dt, tag):
            return con.tile(shape, dt, tag=tag, name=tag)

        # ---- constants / weights (K/V path first so PE starts early) ----
        srcu = [ctile([128, sup], BF16, f"srcu{i}") for i in range(4)]
        wq_sb = [ctile([128, 256], BF16, f"wq{i}") for i in range(4)]
        wk_sb = [ctile([128, 256], BF16, f"wk{i}") for i in range(4)]
        wv_sb = [ctile([128, 256], BF16, f"wv{i}") for i in range(4)]
        for i in range(4):
            dma(wk_sb[i], wk_d[128 * i:128 * (i + 1), :])
            dma(srcu[i], srcu_d[128 * i:128 * (i + 1), :])
        for i in range(4):
            dma(wv_sb[i], wv_d[128 * i:128 * (i + 1), :])
        src_bf = [ctile([128, S], BF16, f"srcbf{i}") for i in range(4)]
        for i in range(4):
            dma(src_bf[i], src_bf_d[128 * i:128 * (i + 1), :])
            dma(wq_sb[i], wq_d[128 * i:128 * (i + 1), :])
        src_res = [ctile([128, R], F32, f"srcres{i}") for i in range(4)]

        def load_src_res():
            for i in range(4):
                dma(src_res[i], src_res_d[128 * i:128 * (i + 1), :])
        wo_sb = [ctile([128, 512], BF16, f"wo{j}") for j in range(4)]
        w1_sb = [ctile([128, PF], BF16, f"w1{i}") for i in range(4)]
        w2_sb = [ctile([128, 512], BF16, f"w2{i}") for i in range(16)]

        def load_chain_weights():
            for j in range(4):
                dma(wo_sb[j], wo_d[128 * j:128 * (j + 1), :])
            for i in range(4):
                dma(w1_sb[i], w1_d[128 * i:128 * (i + 1), :])
            for i in range(16):
                dma(w2_sb[i], w2_d[128 * i:128 * (i + 1), :])

        def vec_in(dram, n, tag):
            t = ctile([128, n], F32, tag)
            dma(t, dram.rearrange("m p -> p m"))
            return t

        bq_sb = vec_in(bq_d, 2, "bq")
        bk_sb = vec_in(bk_d, 2, "bk")
        bo_sb = vec_in(bo_d, 4, "bo")
        b1_sb = vec_in(b1_d, 16, "b1")
        b2_sb = vec_in(b2_d, 4, "b2")
        g1_sb = vec_in(g1_d, 4, "g1")
        bt1_sb = vec_in(bt1_d, 4, "bt1")
        g2_sb = vec_in(g2_d, 4, "g2")
        bt2_sb = vec_in(bt2_d, 4, "bt2")
        mb_sb = vec_in(mb_d, KT, "mb")
        bv_rep = ctile([128, 256], F32, "bvrep")
        dma(bv_rep, _bcast_ap(bv_d, 128))

        ones_bf = ctile([128, 128], BF16, "onesbf")
        dve.memset(ones_bf, 1.0)
        qones = ctile([4, 64], BF16, "qones")
        dve.memset(qones, 0.25)
        eps_t = ctile([128, 1], F32, "eps")
        dve.memset(eps_t, LN_EPS)

        # ---- Q / K projections (output transposed [256, seq] bf16) ----
        q_sb = [ctile([128, S], BF16, f"q{m}") for m in range(2)]
        k_sb = [ctile([128, sup], BF16, f"k{m}") for m in range(2)]

        def proj_mt(w_sb, bias_sb, rhs_at, n_total, out_tiles, mt):
            n0 = 0
            while n0 < n_total:
                nq = min(512, n_total - n0)
                ps = ppool.tile([128, 512], F32, tag="ps", name="ps")
                for ct in range(4):
                    mm(ps[:, :nq], w_sb[ct][:, 128 * mt:128 * (mt + 1)],
                       rhs_at(ct, n0, nq), start=(ct == 0), stop=(ct == 3))
                dve.tensor_scalar_add(out_tiles[mt][:, n0:n0 + nq], ps[:, :nq],
                                      bias_sb[:, mt:mt + 1])
                n0 += nq

        def srcu_at(ct, n0, nq):
            return srcu[ct][:, n0:n0 + nq]

        def srcbf_at(ct, n0, nq):
            return src_bf[ct][:, n0:n0 + nq]

        proj_mt(wk_sb, bk_sb, srcu_at, sup, k_sb, 0)

        # ---- V natural [keys, 4*65] with ones column ----
        v_sb = ctile([128, KT * 4 * 68], BF16, "v")
        v_v = v_sb.rearrange("p (kt h e) -> p kt h e", kt=KT, h=4)
        dve.memset(v_v[:, :, :, 64:68], 1.0)
        for kt in range(KT):
            ps = ppool.tile([128, 512], F32, tag="ps", name="ps")
            for ct in range(4):
                mm(ps[:, :256], srcu[ct][:, 128 * kt:128 * (kt + 1)],
                   wv_sb[ct], start=(ct == 0), stop=(ct == 3))
            dve.tensor_tensor(
                out=v_v[:, kt, :, 0:64],
                in0=ps[:, :256].rearrange("p (h d) -> p h d", h=4),
                in1=bv_rep.rearrange("p (h d) -> p h d", h=4),
                op=OP.add)

        proj_mt(wq_sb, bq_sb, srcbf_at, S, q_sb, 0)

        # ---- attention for head h -> writes its half of pair tile xnp ----
        dens_pool = ctx.enter_context(tc.tile_pool(name="dens", bufs=2))

        def attention(h, xnp):
            g = h // 2
            p0 = 64 * (h % 2)
            o0 = S * (h % 2)
            xp = xp_pool.tile([64, S], F32, tag="xp", name="xp")
            for q0 in range(0, S, 1024):
                avs = []
                for half in range(2):
                    avs.append(pe_av.tile([68, 512], F32, tag="av", name="av"))
                for kt in range(KT):
                    e = pe_e.tile([128, 1024], F32, tag="e", name="e")
                    for half in range(2):
                        mm(e[:, 512 * half:512 * (half + 1)],
                           k_sb[g][p0:p0 + 64, 128 * kt:128 * (kt + 1)],
                           q_sb[g][p0:p0 + 64, q0 + 512 * half:q0 + 512 * (half + 1)],
                           start=True, stop=True)
                    at = att_pool.tile([128, 1024], BF16, tag="att", name="att")
                    act(at, e, AF.Exp, bias=mb_sb[:, kt:kt + 1], scale=1.0)
                    for half in range(2):
                        mm(avs[half], v_v[:, kt, h, :],
                           at[:, 512 * half:512 * (half + 1)],
                           start=(kt == 0), stop=(kt == KT - 1),
                           skip_group_check=True)
                dq = dens_pool.tile([4, 1024], BF16, tag="dens", name="dens")
                for half in range(2):
                    dve.tensor_copy(out=dq[0:4, 512 * half:512 * (half + 1)],
                                    in_=avs[half][64:68, :])
                for half in range(2):
                    dve.tensor_copy(out=xp[:, q0 + 512 * half:q0 + 512 * (half + 1)],
                                    in_=avs[half][0:64, :])
                # normalize: xn = xp * recip(den); den broadcast via K=4 matmul
                rep = rep_pool.tile([64, 1024], F32, tag="rep", name="rep")
                for half in range(2):
                    bc = pe_av.tile([64, 512], F32, tag="av", name="av")
                    mm(bc, qones, dq[0:4, 512 * half:512 * (half + 1)],
                       start=True, stop=True)
                    dve.reciprocal(rep[:, 512 * half:512 * (half + 1)], bc)
                nc.gpsimd.tensor_tensor(out=xnp[0:64, o0 + q0:o0 + q0 + 1024],
                                        in0=xp[:, q0:q0 + 1024],
                                        in1=rep, op=OP.mult)
            # shifted duplicate (partition 64+d, col f) = (d, f+1): lets Wo read
            # both 64-row c-blocks of a j-pair as one K=128 operand
            nc.gpsimd.tensor_copy(out=xnp[64:128, o0:o0 + S - 1],
                                  in_=xnp[0:64, o0 + 1:o0 + S])

        # ---- layernorm on a 512-row pair block: z_tiles 4x[128,512] fp32 ----
        def layernorm(z_tiles, g_sb, b_sb, writers, W=512):
            s1 = ppool.tile([128, W], F32, tag="ps", name="ps")
            s2 = ppool.tile([128, W], F32, tag="ps", name="ps")
            for ct in range(4):
                zb = tmp_pool.tile([128, W], BF16, tag="zb", name="zb")
                nc.gpsimd.tensor_copy(out=zb, in_=z_tiles[ct])
                sq = tmp_pool.tile([128, W], BF16, tag="sq", name="sq")
                nc.gpsimd.tensor_mul(sq, z_tiles[ct], z_tiles[ct])
                mm(s1, ones_bf, zb, start=(ct == 0), stop=(ct == 3),
                   skip_group_check=True)
                mm(s2, ones_bf, sq, start=(ct == 0), stop=(ct == 3),
                   skip_group_check=True)
            bm = tmp_pool.tile([128, W], F32, tag="bm", name="bm")
            br = tmp_pool.tile([128, W], F32, tag="br", name="br")
            m2 = tmp_pool.tile([128, W], F32, tag="m2", name="m2", bufs=1)
            dve.tensor_scalar_mul(bm, s1, 1.0 / HID)                # mean
            dve.tensor_mul(m2, bm, bm)                              # mean^2
            dve.scalar_tensor_tensor(out=br, in0=s2,
                                     scalar=1.0 / HID, in1=m2,
                                     op0=OP.mult, op1=OP.subtract)  # var
            act(br, br, AF.Ln, bias=eps_t)
            act(br, br, AF.Exp, scale=-0.5)                         # rstd
            for ct in range(4):
                sub = tmp_pool.tile([128, W], F32, tag="sub", name="sub")
                dve.tensor_tensor(out=sub, in0=z_tiles[ct], in1=bm, op=OP.subtract)
                t2 = tmp_pool.tile([128, W], F32, tag="t2", name="t2")
                dve.scalar_tensor_tensor(out=t2, in0=sub,
                                         scalar=g_sb[:, ct:ct + 1], in1=br,
                                         op0=OP.mult, op1=OP.mult)
                writers(ct, t2, b_sb)

        # ---- per-pair chain: Wo + residual, LN1, FFN, LN2, out (512 rows) ----
        # src1 tiles reuse the SBUF slots of src_bf/srcu (dead after projections)
        src1_f = [[con.tile([128, 512], F32, tag=f"srcbf{i}",
                             name=f"s1f{i}_{j}") for j in (0, 1)]
                  for i in range(4)]
        src1_b = [con.tile([128, R], BF16, tag=f"srcu{i}", name=f"s1b{i}")
                  for i in range(4)]

        def chain_a(hp, xnp):
            c0 = 512 * hp
            xv = xnp.rearrange("p (hh m j) -> p j hh m", hh=2, j=8)
            z1 = [z_pool.tile([128, 512], F32, tag=f"z{mt}", name=f"z{mt}")
                  for mt in range(4)]
            for mt in range(4):
                ps = ppool.tile([128, 512], F32, tag="ps", name="ps")
                for jp in range(4):
                    mm(ps, wo_sb[jp][:, 128 * mt:128 * (mt + 1)],
                       xv[:, 2 * jp], start=(jp == 0), stop=(jp == 3))
                dve.scalar_tensor_tensor(out=z1[mt], in0=ps,
                                         scalar=bo_sb[:, mt:mt + 1],
                                         in1=src_res[mt][:, c0:c0 + 512],
                                         op0=OP.add, op1=OP.add)

            def w1(ct, t2, b_sb):
                act(src1_f[ct][c0 // 512], t2, AF.Identity,
                    bias=b_sb[:, ct:ct + 1], scale=1.0)
                nc.gpsimd.tensor_scalar_add(src1_b[ct][:, c0:c0 + 512], t2,
                                            b_sb[:, ct:ct + 1])

            layernorm(z1, g1_sb, bt1_sb, w1)

        def chain_b(hp, split_tail=False):
            c0 = 512 * hp
            h1 = h1_pool.tile([128, 16 * 512], BF16, tag="h1", name="h1", bufs=1)
            for mt in range(16):
                ps = ppool.tile([128, 512], F32, tag="ps", name="ps")
                for ct in range(4):
                    mm(ps, w1_sb[ct][:, 128 * mt:128 * (mt + 1)],
                       src1_b[ct][:, c0:c0 + 512], start=(ct == 0), stop=(ct == 3))
                if mt % 2 == 0:
                    act(h1[:, 512 * mt:512 * (mt + 1)], ps, AF.Relu,
                        bias=b1_sb[:, mt:mt + 1], scale=1.0)
                else:
                    dve.tensor_scalar(out=h1[:, 512 * mt:512 * (mt + 1)], in0=ps,
                                      scalar1=b1_sb[:, mt:mt + 1], scalar2=0.0,
                                      op0=OP.add, op1=OP.max)

            halves = ((0, 512),) if not split_tail else ((0, 256), (256, 256))
            for r0, W in halves:
                z2 = [z_pool.tile([128, 512], F32, tag=f"z{ot}", name=f"z{ot}")
                      for ot in range(4)]
                for ot in range(4):
                    ps = ppool.tile([128, 512], F32, tag="ps", name="ps")
                    for mt in range(16):
                        mm(ps[:, :W], w2_sb[mt][:, 128 * ot:128 * (ot + 1)],
                           h1[:, 512 * mt + r0:512 * mt + r0 + W],
                           start=(mt == 0), stop=(mt == 15))
                    dve.scalar_tensor_tensor(
                        out=z2[ot][:, :W], in0=ps[:, :W],
                        scalar=b2_sb[:, ot:ot + 1],
                        in1=src1_f[ot][c0 // 512][:, r0:r0 + W],
                        op0=OP.add, op1=OP.add)

                def w2(ct, t2, b_sb, r0=r0, W=W):
                    o = o_pool.tile([128, 512], F32, tag="out", name="out", bufs=2)
                    act(o[:, :W], t2, AF.Identity, bias=b_sb[:, ct:ct + 1],
                        scale=1.0)
                    dma(out_d[128 * ct:128 * (ct + 1), c0 + r0:c0 + r0 + W],
                        o[:, :W])

                layernorm([z[:, :W] for z in z2], g2_sb, bt2_sb, w2, W=W)

        # ---- schedule ----
        xnp0 = xn_pool.tile([128, 2 * S], BF16, tag="xn", name="xn")
        attention(0, xnp0)
        load_chain_weights()
        load_src_res()
        proj_mt(wk_sb, bk_sb, srcu_at, sup, k_sb, 1)
        proj_mt(wq_sb, bq_sb, srcbf_at, S, q_sb, 1)
        attention(1, xnp0)
        xnp1 = xn_pool.tile([128, 2 * S], BF16, tag="xn", name="xn")
        attention(2, xnp1)
        chain_a(0, xnp0)
        attention(3, xnp1)
        chain_b(0, split_tail=True)
        chain_a(1, xnp1)
        chain_b(1, split_tail=True)

    nc.compile()
    return nc


def _prep_core(c, src, idxs, sup, w):
    b, hg = c // 2, c % 2
    heads = list(range(HPC * hg, HPC * hg + HPC))
    bf = ml_dtypes.bfloat16
    st = np.ascontiguousarray(src[b].T)                       # [512, 2048] f32
    idx = idxs[b]
    su = len(idx)
    srcu = np.zeros((HID, sup), np.float32)
    srcu[:, :su] = st[:, idx]
    wqe = np.concatenate([w["Wm"] @ w["Wq"][64 * h:64 * (h + 1), :] for h in heads])
    bqe = np.concatenate([w["Wm"] @ w["bq"][64 * h:64 * (h + 1)] + w["bm"]
                          for h in heads])
    wks = np.concatenate([w["Wk"][64 * h:64 * (h + 1), :] for h in heads])
    bks = np.concatenate([w["bk"][64 * h:64 * (h + 1)] for h in heads])
    wvs = np.concatenate([w["Wv"][64 * h:64 * (h + 1), :] for h in heads])
    bvs = np.concatenate([w["bv"][64 * h:64 * (h + 1)] for h in heads])
    mb = np.full(sup, NEG_BIG, np.float32)
    mb[:su] = 0.0
    f32 = np.float32
    return {
        "src_bf": st.astype(bf),
        "src_res": np.ascontiguousarray(st[:, R * hg:R * (hg + 1)]),
        "srcu": srcu.astype(bf),
        "wq": np.ascontiguousarray(wqe.T).astype(bf),
        "wk": np.ascontiguousarray(wks.T).astype(bf),
        "wv": np.ascontiguousarray(wvs.T).astype(bf),
        "wo": np.ascontiguousarray(w["Wo"].T).astype(bf),
        "w1": np.ascontiguousarray(w["W1"].T).astype(bf),
        "w2": np.ascontiguousarray(w["W2"].T).astype(bf),
        "bq": bqe.reshape(2, 128).astype(f32),
        "bk": bks.reshape(2, 128).astype(f32),
        "bv": bvs.astype(f32),
        "bo": w["bo"].reshape(4, 128).astype(f32),
        "b1": w["b1"].reshape(16, 128).astype(f32),
        "b2": w["b2"].reshape(4, 128).astype(f32),
        "g1": w["ln1_g"].reshape(4, 128).astype(f32),
        "bt1": w["ln1_b"].reshape(4, 128).astype(f32),
        "g2": w["ln2_g"].reshape(4, 128).astype(f32),
        "bt2": w["ln2_b"].reshape(4, 128).astype(f32),
        "mb": mb.reshape(sup // 128, 128),
    }


def kernel(**inputs):
    global last_results
    w = {k: np.asarray(v, np.float32) for k, v in inputs.items()
         if k not in ("src", "src_mask")}
    src = np.asarray(inputs["src"], np.float32)
    mask = np.asarray(inputs["src_mask"]).reshape(B, S)
    idxs = [np.nonzero(mask[b] != 0)[0] for b in range(B)]
    sup = max(128, ((max(len(i) for i in idxs) + 127) // 128) * 128)

    if sup not in _built_cache:
        _built_cache[sup] = build_bass(sup)
    nc = _built_cache[sup]

    in_maps = [_prep_core(c, src, idxs, sup, w) for c in range(N_CORES)]
    res = bass_utils.run_bass_kernel_spmd(nc, in_maps, core_ids=list(range(N_CORES)),
                                          **run_kwargs)
    last_results = res
    out = np.empty((B, S, HID), np.float32)
    for c in range(N_CORES):
        b, hg = c // 2, c % 2
        out[b, R * hg:R * (hg + 1), :] = res.results[c]["out_t"].T
    return out

